# revision 2
# baseline (speedup 1.0000x reference)
"""Trainium2 Bass kernel for the SPH composition loss — poly-spline v2.

Query-major layout: each of 48 Morton blocks owns 128 query rows
(partitions); its exact h-ball candidate union lies along the free axis.
Per block, two fp32r GEMMs produce d2 and the pair dot-product in PSUM.
All per-pair math is evaluated as polynomials in x = d2/h^2 (clamped to
[*,1]); the cubic-spline W and dW terms are least-squares-fitted
polynomials that vanish exactly at x=1, so masking and candidate padding
cost nothing and there are no transcendentals, divisions, or NaN paths.

  W-side:  S(x) ~= a0+a1x+a2x^2+a3x^3+a4x^4   (sum_j W = -2*sigma*sum S)
           sums via two affine_mul_reduce: (a3x^2+a1)*x and (a4x^2+a2)*x^2
  div-side: Phi(x) = G/q ~= (x-1)*b2*((x-m)^2+c);  div = C*sum Phi*dot
           one fused multiply+accum against the dot PSUM per chunk.

Per-core partial sums are combined on host (the all-reduce of the three
scalar loss means).
"""
import sys
sys.path.insert(0, "/opt/trn_rl_repo")
import numpy as np
from contextlib import ExitStack

NCORES = 8
BQ = 128
GRID = 9
CHUNK = 512
GRAN = 16

# spline fits in x = q^2 on [0,1], exact zero at x=1 (see module docstring)
A0, A1, A2, A3, A4 = (-0.48853911110105286, 2.326741741336326,
                      -4.375041250395634, 3.7772893760790103,
                      -1.2404507559186488)
PB2 = -2.8931258640379087     # psi = PB2*((x-PM)^2 + PC); Phi=(x-1)*psi
PM = 0.7098721396623986
PC = 0.029900487445892687

_PROGRAM_CACHE = {}
_last_results = None
OPTS = {
    "xc_eng": "vector",    # psd2 -> xc drain (mult,min)
    "rt_eng": "vector",    # psdot*phi accumulate
    "psi_eng": "gpsimd",   # f2 + PC
    "u1_eng": "scalar",    # b2*(x-1) via Copy
    "phi_eng": "gpsimd",   # u1*psi
    "x2_eng": "scalar",    # Square
    "f2_eng": "scalar",    # Square(x - PM)
    "dma": "sync",
    "ps_bufs": 3,
    "wp_bufs": 2,
}


# ---------------------------------------------------------------- host prep
def _morton3(c):
    out = np.zeros(len(c), dtype=np.int64)
    for b in range(4):
        for d in range(3):
            out |= ((c[:, d] >> b) & 1) << (3 * b + d)
    return out


def _build_structure(pos, h):
    """Morton sort + per-block exact candidate lists (ball union)."""
    N = pos.shape[0]
    cell = np.clip(np.floor(pos * GRID).astype(np.int64), 0, GRID - 1)
    perm = np.argsort(_morton3(cell), kind="stable")
    pos_s = pos[perm]
    nblk = N // BQ
    cand_lists = []
    from scipy.spatial import cKDTree
    tree_all = cKDTree(pos_s)
    for b in range(nblk):
        qt = cKDTree(pos_s[b * BQ:(b + 1) * BQ])
        idx = qt.query_ball_tree(tree_all, r=float(h) * (1 + 1e-6))
        s = set()
        for lst in idx:
            s.update(lst)
        cand_lists.append(np.array(sorted(s), dtype=np.int64))
    return perm, cand_lists


# ---------------------------------------------------------------- program
def _build_program(wk_list, h, vol, reps=1):
    import concourse.bass as bass
    import concourse.tile as tile
    from concourse import bacc, mybir, bass_isa
    from concourse.alu_op_type import AluOpType as alu

    f32 = mybir.dt.float32
    f16 = mybir.dt.float16
    f32r = mybir.dt.float32r
    AF = mybir.ActivationFunctionType

    h = float(h)
    vol = float(vol)
    sigma = 8.0 / (np.pi * h ** 3)
    inv_h2 = 1.0 / (h * h)
    KAPPA2 = -2.0 * sigma * vol          # rho/rho0 = KAPPA2 * sumS
    CDIV = -6.0 * sigma * vol / h / h    # div = CDIV * sum Phi*dot

    NB = len(wk_list)
    NQ = NB * BQ
    W = sum(wk_list)
    l1w = (NQ * 3) // 128
    wk_max = max(wk_list)
    # chunks: (block, col_off_in_block, width, global_chunk_idx)
    chunks = []
    for k, wk in enumerate(wk_list):
        o = 0
        while o < wk:
            cw = min(CHUNK, wk - o)
            chunks.append((k, o, cw, len(chunks)))
            o += cw
    nch = len(chunks)

    nc = bacc.Bacc("TRN2", target_bir_lowering=False, debug=False,
                   num_devices=NCORES)

    d_lhs2 = nc.dram_tensor("lhs2", [5, NB * BQ], f32,
                            kind="ExternalInput").ap()
    d_lhs8 = nc.dram_tensor("lhs8", [8, NB * BQ], f32,
                            kind="ExternalInput").ap()
    d_rhs2 = nc.dram_tensor("rhs2", [5, W], f32, kind="ExternalInput").ap()
    d_rhs8 = nc.dram_tensor("rhs8", [8, W], f32, kind="ExternalInput").ap()
    d_ypred = nc.dram_tensor("ypred", [128, 2 * l1w], f32,
                             kind="ExternalInput").ap()
    d_cw = nc.dram_tensor("cw", [128, 16], f32, kind="ExternalInput").ap()
    d_out = nc.dram_tensor("out", [1, 4], f32, kind="ExternalOutput").ap()

    es = ExitStack()
    with tile.TileContext(nc) as tc:
        with es:
            pin = es.enter_context(tc.tile_pool(name="pin", bufs=1))
            wp = es.enter_context(tc.tile_pool(name="wp",
                                               bufs=OPTS["wp_bufs"]))
            acc = es.enter_context(tc.tile_pool(name="acc", bufs=1))
            tail = es.enter_context(tc.tile_pool(name="tail", bufs=1))
            ps2 = es.enter_context(tc.tile_pool(
                name="ps2", bufs=2, space=bass.MemorySpace.PSUM))
            psd = es.enter_context(tc.tile_pool(
                name="psd", bufs=OPTS["ps_bufs"], space=bass.MemorySpace.PSUM))

            dmae = getattr(nc, OPTS["dma"])
            xc_eng = getattr(nc, OPTS["xc_eng"])
            rt_eng = getattr(nc, OPTS["rt_eng"])
            psi_eng = getattr(nc, OPTS["psi_eng"])
            phi_eng = getattr(nc, OPTS["phi_eng"])

            rhs2_sb = pin.tile([5, W], f32, tag="rhs2")
            dmae.dma_start(rhs2_sb[:], d_rhs2)
            lhs2_sb = pin.tile([5, NB * BQ], f32, tag="lhs2")
            dmae.dma_start(lhs2_sb[:], d_lhs2)
            lhs8_sb = pin.tile([8, NB * BQ], f32, tag="lhs8")
            dmae.dma_start(lhs8_sb[:], d_lhs8)
            cw_sb = pin.tile([128, 16], f32, tag="cw")
            dmae.dma_start(cw_sb[:], d_cw)
            C_NPM = cw_sb[:, 6:7]
            C_PB2 = cw_sb[:, 7:8]
            C_NB2 = cw_sb[:, 8:9]
            C_K2 = cw_sb[:, 9:10]
            C_N1 = cw_sb[:, 10:11]
            C_CD = cw_sb[:, 11:12]
            rhs8_sb = pin.tile([8, W], f32, tag="rhs8")
            nc.gpsimd.dma_start(rhs8_sb[:], d_rhs8)
            ypred_sb = pin.tile([128, 2 * l1w], f32, tag="ypred")
            nc.gpsimd.dma_start(ypred_sb[:], d_ypred)

            rS = acc.tile([128, NB], f32, tag="rS")
            rH = acc.tile([128, NB], f32, tag="rH")
            rTc = acc.tile([128, nch], f32, tag="rTc")
            out_sb = tail.tile([1, 4], f32, tag="osb")
            nc.gpsimd.memset(out_sb[:], 0.0)

            for rep in range(reps):
                nc.scalar.activation(out_sb[0:1, 3:4], out_sb[0:1, 3:4],
                                     AF.Identity, bias=1.0)
                boff = 0
                for k, wk in enumerate(wk_list):
                    lq2 = lhs2_sb[:, k * BQ:(k + 1) * BQ]
                    lq8 = lhs8_sb[:, k * BQ:(k + 1) * BQ]
                    xc = wp.tile([128, wk_max], f16, tag="xc")
                    blk_chunks = [c for c in chunks if c[0] == k]
                    psdots = []
                    for (_, o, cwd, ci) in blk_chunks:
                        rr = slice(boff + o, boff + o + cwd)
                        psd2 = ps2.tile([128, cwd], f32, tag="psd2")
                        nc.tensor.matmul(psd2[:], lq2, rhs2_sb[:, rr],
                                         start=True, stop=True)
                        pdot = psd.tile([128, cwd], f32, tag="psdot")
                        nc.tensor.matmul(pdot[:], lq8, rhs8_sb[:, rr],
                                         start=True, stop=True)
                        psdots.append((pdot, o, cwd, ci))
                        xc_eng.tensor_scalar(xc[:, o:o + cwd], psd2[:],
                                             inv_h2, 1.0, alu.mult, alu.min)
                    x2 = wp.tile([128, wk_max], f16, tag="x2")
                    nc.scalar.activation(x2[:, :wk], xc[:, :wk], AF.Square)
                    f2 = wp.tile([128, wk_max], f16, tag="f2")
                    nc.scalar.activation(f2[:, :wk], xc[:, :wk], AF.Square,
                                         bias=C_NPM)
                    u1 = wp.tile([128, wk_max], f16, tag="u1")
                    nc.scalar.activation(u1[:, :wk], xc[:, :wk], AF.Copy,
                                         bias=-PB2, scale=PB2)
                    scrS = wp.tile([128, wk_max], f16, tag="scrS")
                    nc.vector.affine_mul_reduce(
                        scrS[:, :wk], rS[:, k:k + 1], x2[:, :wk], xc[:, :wk],
                        A3, A1)
                    scrH = wp.tile([128, wk_max], f16, tag="scrH")
                    nc.vector.affine_mul_reduce(
                        scrH[:, :wk], rH[:, k:k + 1], x2[:, :wk], x2[:, :wk],
                        A4, A2)
                    psi = wp.tile([128, wk_max], f16, tag="psi")
                    psi_eng.tensor_scalar(psi[:, :wk], f2[:, :wk], PC, None,
                                          alu.add)
                    phi = wp.tile([128, wk_max], f16, tag="phi")
                    phi_eng.tensor_tensor(phi[:, :wk], u1[:, :wk],
                                          psi[:, :wk], alu.mult)
                    scrT = wp.tile([128, CHUNK], f16, tag="scrT")
                    for (pdot, o, cwd, ci) in psdots:
                        rt_eng.scalar_tensor_tensor(
                            scrT[:, :cwd], pdot[:], 1.0, phi[:, o:o + cwd],
                            alu.bypass, alu.mult,
                            accum_out=rTc[:, ci:ci + 1])
                    boff += wk

            # ---- final combines ----
            fin = tail.tile([128, 4], f32, tag="fin")
            Tall = tail.tile([128, NB], f32, tag="Tall")
            ci0 = 0
            for k, wk in enumerate(wk_list):
                ncb = len([c for c in chunks if c[0] == k])
                nc.vector.tensor_reduce(Tall[:, k:k + 1],
                                        rTc[:, ci0:ci0 + ncb],
                                        mybir.AxisListType.X, alu.add)
                ci0 += ncb
            Sall = tail.tile([128, NB], f32, tag="Sall")
            nc.vector.tensor_tensor(Sall[:], rS[:], rH[:], alu.add)
            Sadj = tail.tile([128, NB], f32, tag="Sadj")
            nc.vector.tensor_tensor(Sadj[:], Sall[:], cw_sb[:, 0:NB], alu.add)
            z2 = tail.tile([128, NB], f32, tag="z2")
            nc.scalar.activation(z2[:], Sadj[:], AF.Abs, bias=C_N1,
                                 scale=C_K2, accum_out=fin[:, 1:2])
            z3 = tail.tile([128, NB], f32, tag="z3")
            nc.scalar.activation(z3[:], Tall[:], AF.Abs, scale=C_CD,
                                 accum_out=fin[:, 2:3])
            # loss1
            e_t = tail.tile([128, l1w], f32, tag="e")
            nc.vector.tensor_tensor(e_t[:], ypred_sb[:, 0:l1w],
                                    ypred_sb[:, l1w:2 * l1w], alu.subtract)
            esq = tail.tile([128, l1w], f32, tag="esq")
            nc.scalar.activation(esq[:], e_t[:], AF.Square,
                                 accum_out=fin[:, 0:1])
            nc.gpsimd.memset(fin[:, 3:4], 0.0)
            finpr = tail.tile([128, 4], f32, tag="finpr")
            nc.gpsimd.partition_all_reduce(finpr[:], fin[:], 128,
                                           bass_isa.ReduceOp.add)
            nc.scalar.activation(out_sb[0:1, 0:3], finpr[0:1, 0:3], AF.Copy)
            nc.sync.dma_start(d_out, out_sb[:])
    nc.compile()
    return nc


# ---------------------------------------------------------------- kernel
def prepare(inputs, reps=1):
    pred = np.asarray(inputs["pred"], dtype=np.float64)
    y = np.asarray(inputs["y"], dtype=np.float64)
    mid_pos = np.asarray(inputs["mid_pos"], dtype=np.float64)
    mid_vel = np.asarray(inputs["mid_vel"], dtype=np.float64)
    y_mean = np.asarray(inputs["y_mean"], dtype=np.float64)
    y_std = np.asarray(inputs["y_std"], dtype=np.float64)
    h = float(inputs["h"])
    vol = float(inputs["vol"])
    dt = float(inputs["dt"])
    nb = int(inputs["num_boundary_particles"])
    N = pred.shape[0]
    rows_core = N // NCORES

    y_inv = y * y_std + y_mean
    pos = mid_pos.copy()
    pos[nb:] += y_inv[nb:]
    vel = mid_vel.copy()
    vel[nb:] += y_inv[nb:] / dt

    perm, cand_lists = _build_structure(pos.astype(np.float32), h)
    pos_s = pos[perm]
    vel_s = vel[perm]
    y_s = y[perm].astype(np.float32)
    pred_s = pred[perm].astype(np.float32)

    nblk_total = N // BQ
    nblk_core = nblk_total // NCORES
    order = np.argsort([-len(c) for c in cand_lists], kind="stable")
    slots = [order[k * NCORES:(k + 1) * NCORES] for k in range(nblk_core)]
    wk_list = []
    for k in range(nblk_core):
        mx = max(len(cand_lists[b]) for b in slots[k])
        wk_list.append(int(np.ceil(mx / GRAN)) * GRAN)

    key = (tuple(wk_list), float(h), float(vol), N, reps)
    if key not in _PROGRAM_CACHE:
        _PROGRAM_CACHE[key] = _build_program(wk_list, h, vol, reps=reps)
    nc = _PROGRAM_CACHE[key]

    W = sum(wk_list)
    l1w = (rows_core * 3) // 128

    in_maps = []
    for c in range(NCORES):
        lhs = np.empty((13, nblk_core * BQ), np.float64)
        rhs = np.empty((13, W), np.float64)
        # rows 0-4: d2 factors; rows 5-12: dot factors (split on output)
        qsel = []
        off = 0
        for k in range(nblk_core):
            b = int(slots[k][c])
            qidx = np.arange(b * BQ, (b + 1) * BQ)
            qsel.append(qidx)
            ci = cand_lists[b]
            wk = wk_list[k]
            npad = wk - len(ci)
            center = pos_s[qidx].mean(axis=0)
            qp = pos_s[qidx] - center
            qv = vel_s[qidx]
            qsq = (qp * qp).sum(1)
            qdiag = (qp * qv).sum(1)
            cp = np.concatenate([pos_s[ci] - center,
                                 np.full((npad, 3), 50.0)])
            cv = np.concatenate([vel_s[ci], np.zeros((npad, 3))])
            csq = (cp * cp).sum(1)
            cdiag = (cp * cv).sum(1)
            ql = slice(k * BQ, (k + 1) * BQ)
            lhs[0:3, ql] = qp.T
            lhs[3, ql] = qsq
            lhs[4, ql] = 1.0
            lhs[5:8, ql] = qv.T
            lhs[8:11, ql] = qp.T
            lhs[11, ql] = 1.0
            lhs[12, ql] = qdiag
            cl = slice(off, off + wk)
            rhs[0:3, cl] = -2.0 * cp.T
            rhs[3, cl] = 1.0
            rhs[4, cl] = csq
            rhs[5:8, cl] = cp.T
            rhs[8:11, cl] = cv.T
            rhs[11, cl] = -cdiag
            rhs[12, cl] = -1.0
            off += wk
        lhsf = lhs.astype(np.float32)
        rhsf = rhs.astype(np.float32)
        m = {"lhs2": lhsf[0:5], "lhs8": lhsf[5:13],
             "rhs2": rhsf[0:5], "rhs8": rhsf[5:13]}
        qidx = np.concatenate(qsel)
        ypred = np.empty((128, 2 * l1w), np.float32)
        ypred[:, 0:l1w] = y_s[qidx].reshape(128, l1w)
        ypred[:, l1w:] = pred_s[qidx].reshape(128, l1w)
        m["ypred"] = ypred
        cw = np.zeros((128, 16), np.float32)
        for k in range(nblk_core):
            cw[:, k] = A0 * wk_list[k]
        sigma = 8.0 / (np.pi * h ** 3)
        cw[:, 6] = -PM
        cw[:, 7] = PB2
        cw[:, 8] = -PB2
        cw[:, 9] = -2.0 * sigma * vol
        cw[:, 10] = -1.0
        cw[:, 11] = -6.0 * sigma * vol / h / h
        m["cw"] = cw
        in_maps.append(m)
    return nc, in_maps, N


def combine(results, N):
    parts = np.stack([results[c]["out"][0] for c in range(NCORES)])
    l1 = float(np.sum(parts[:, 0], dtype=np.float64))
    l2 = float(np.sum(parts[:, 1], dtype=np.float64))
    l3 = float(np.sum(parts[:, 2], dtype=np.float64))
    total = np.float32(1.0 * l1 / N) + np.float32(0.1) * np.float32(l2 / N) \
        + np.float32(0.1) * np.float32(l3 / N)
    return np.array(total, dtype=np.float32)


def kernel(**inputs):
    from concourse.bass_utils import run_bass_kernel_spmd
    nc, in_maps, N = prepare(inputs)
    res = run_bass_kernel_spmd(nc, in_maps, core_ids=list(range(NCORES)))
    global _last_results
    _last_results = res
    return combine(res.results, N)
